# revision 7
# baseline (speedup 1.0000x reference)
"""AttnBlock (GroupNorm + single-head spatial attention + residual) on 8 trn2 cores.

Sharding: 8 cores = 4 batches x 2 query-halves. The host pre-rolls the spatial
axis per core so the SPMD program always works on query columns 0:2048; k/v and
GroupNorm run over all 4096 positions (softmax/GN are permutation invariant).
No cross-core communication is needed.

Per-core layout is channels-on-partitions [C, N] throughout, which makes every
stage a plain TensorE contraction with no transposes:
  h   = GN(x)                      [C, N]   (stats via DVE/ACT + selector matmuls)
  q/k = Wq/Wk @ h  (+bias in ACT)  [C, Nq] / [C, N]
  v   = h.T-tiles @ WvT (+bias mm) [N, C]   (computed directly in N-major layout)
  sT  = k.T-tiles @ q              [Nk, Nq] accumulated over C
  p   = exp(sT / sqrt(C))          (no max subtraction; scores are ~N(0,1))
  den = ones.T @ p                 [1, Nq]  (partition-sum via matmul)
  o   = v-tiles.T @ p              [C, Nq]  accumulated over Nk, normalized by 1/den
  y   = x + Wo @ o + bo            [C, Nq]
All heavy matmuls run in bf16 (validated: ~7e-4 rel err vs fp32 reference).
"""

import functools
import os
import sys

sys.path.insert(0, "/opt/trn_rl_repo")

import numpy as np
import ml_dtypes

BF16 = ml_dtypes.bfloat16

C = 512
N = 4096
NQ = 2048
P = 128
CT = 4            # channel tiles of 128
NKT = 32          # nk tiles of 128
QB = 4            # nq blocks of 512
NB = 512          # block width
EPS = 1e-5
SCALE = 1.0 / float(np.sqrt(C))
N_CORES = 8

_LAST_RESULT = None  # test.py reads exec_time_ns from here


def _install_drain_patch():
    """This nix walrus allows max 1 sync-wait per InstDrain; Tile's kernel-tail
    drain carries one wait per active engine/queue. Spread them over a chain of
    drains."""
    import concourse.mybir as mybir
    import concourse.tile as tile
    from concourse.vector_clock import ScopedClock

    if getattr(tile.TileContext, "_drain_patch_installed", False):
        return

    def _patched(self, tick_clock, wait_clock):
        drain_bi = self.nc.sync.drain()
        wait_clock.add_sem_waits(
            drain_bi.ins, ScopedClock({None: tick_clock.global_clock})
        )
        inst = drain_bi.ins
        waits = list(inst.sync_info.on_wait)
        if len(waits) > 1:
            inst.sync_info = mybir.SyncInfo(
                on_wait=waits[:1], on_update=list(inst.sync_info.on_update)
            )
            for w in waits[1:]:
                d2 = self.nc.sync.drain()
                d2.ins.sync_info = mybir.SyncInfo(on_wait=[w], on_update=[])
        self.nc.all_engine_barrier()
        popped = self.nc._tile_sem_poison_stack.pop()
        assert popped is self._sem_poison
        self.nc.clear_and_free_semaphores(list(self.sems.allocated().values()))
        self.nc.all_engine_barrier()

    tile.TileContext._drain_and_barrier = _patched
    tile.TileContext._drain_patch_installed = True


def _legalize_sync_waits(nc, max_waits=1):
    """This walrus build encodes at most one sync-wait per instruction for
    several instruction formats. Hoist extra waits onto same-engine NOPs
    placed immediately before the instruction (same stream position => same
    synchronization semantics)."""
    import concourse.mybir as mybir

    def mk_wait_nop(engine, wait):
        bi = nc.engines[engine].nop(nofuse=True)
        inst = bi.ins
        lst = nc.cur_bb.bb.instructions
        assert lst[-1] is inst
        lst.pop()
        inst.sync_info = mybir.SyncInfo(on_wait=[wait], on_update=[])
        return inst

    n_split = 0
    for fn in nc.m.functions:
        for blk in fn.blocks:
            newlist = []
            for inst in list(blk.instructions):
                si = getattr(inst, "sync_info", None)
                waits = list(si.on_wait) if si is not None else []
                if len(waits) > max_waits:
                    keep = waits[-max_waits:]
                    for w in waits[:-max_waits]:
                        newlist.append(mk_wait_nop(inst.engine, w))
                    inst.sync_info = mybir.SyncInfo(
                        on_wait=keep, on_update=list(si.on_update)
                    )
                    n_split += 1
                newlist.append(inst)
            blk.instructions = newlist
    return n_split


@functools.lru_cache(maxsize=1)
def _build_program():
    import concourse.bass as bass
    import concourse.mybir as mybir
    import concourse.tile as tile

    _install_drain_patch()

    dt = mybir.dt
    AF = mybir.ActivationFunctionType
    ALU = mybir.AluOpType
    AX = mybir.AxisListType

    nc = bass.Bass()

    xf32_d = nc.dram_tensor("xf32", [C, N], dt.float32, kind="ExternalInput")
    xbf_d = nc.dram_tensor("xbf", [C, N], dt.bfloat16, kind="ExternalInput")
    wq_d = nc.dram_tensor("wq", [P, CT, C], dt.bfloat16, kind="ExternalInput")
    wk_d = nc.dram_tensor("wk", [P, CT, C], dt.bfloat16, kind="ExternalInput")
    wv_d = nc.dram_tensor("wv", [P, CT, C], dt.bfloat16, kind="ExternalInput")
    wo_d = nc.dram_tensor("wo", [P, CT, C], dt.bfloat16, kind="ExternalInput")
    bq_d = nc.dram_tensor("bq", [P, CT], dt.float32, kind="ExternalInput")
    bk_d = nc.dram_tensor("bk", [P, CT], dt.float32, kind="ExternalInput")
    bv_d = nc.dram_tensor("bv", [1, C], dt.bfloat16, kind="ExternalInput")
    bo_d = nc.dram_tensor("bo", [1, C], dt.bfloat16, kind="ExternalInput")
    gns_d = nc.dram_tensor("gns", [P, CT], dt.float32, kind="ExternalInput")
    gnb_d = nc.dram_tensor("gnb", [P, CT], dt.float32, kind="ExternalInput")
    sel_d = nc.dram_tensor("sel", [P, 8], dt.float32, kind="ExternalInput")
    selB_d = nc.dram_tensor("selB", [8, P], dt.float32, kind="ExternalInput")
    y_d = nc.dram_tensor("y", [C, NQ], dt.float32, kind="ExternalOutput")

    with tile.TileContext(nc) as tc:
        with (
            tc.tile_pool(name="wpool", bufs=1) as wpool,
            tc.tile_pool(name="xpool", bufs=2) as xpool,
            tc.tile_pool(name="sqpool", bufs=1) as sqpool,
            tc.tile_pool(name="hpool", bufs=4) as hpool,
            tc.tile_pool(name="qpool", bufs=4) as qpool,
            tc.tile_pool(name="kpool", bufs=4) as kpool,
            tc.tile_pool(name="vpool", bufs=32) as vpool,
            tc.tile_pool(name="ppool", bufs=3) as ppool,
            tc.tile_pool(name="opool", bufs=4) as opool,
            tc.tile_pool(name="gpool", bufs=2) as gpool,
            tc.tile_pool(name="chpool", bufs=2) as chpool,
            tc.tile_pool(name="rbpool", bufs=2) as rbpool,
            tc.tile_pool(name="xrpool", bufs=2) as xrpool,
            tc.tile_pool(name="outpool", bufs=2) as outpool,
            tc.tile_pool(name="psA", bufs=2, space="PSUM") as psA,
            tc.tile_pool(name="pden", bufs=1, space="PSUM") as pden,
            tc.tile_pool(name="po", bufs=4, space="PSUM") as po,
        ):
            # ---- constants / weights into SBUF ----
            wq_sb = wpool.tile([P, CT, C], dt.bfloat16, tag="wq")
            wk_sb = wpool.tile([P, CT, C], dt.bfloat16, tag="wk")
            wv_sb = wpool.tile([P, CT, C], dt.bfloat16, tag="wv")
            wo_sb = wpool.tile([P, CT, C], dt.bfloat16, tag="wo")
            for sb, d in ((wq_sb, wq_d), (wk_sb, wk_d), (wv_sb, wv_d), (wo_sb, wo_d)):
                nc.sync.dma_start(sb[:], d[:])
            bq_sb = wpool.tile([P, CT], dt.float32, tag="bq")
            bk_sb = wpool.tile([P, CT], dt.float32, tag="bk")
            gns_sb = wpool.tile([P, CT], dt.float32, tag="gns")
            gnb_sb = wpool.tile([P, CT], dt.float32, tag="gnb")
            sel_sb = wpool.tile([P, 8], dt.float32, tag="sel")
            selB_sb = wpool.tile([8, P], dt.float32, tag="selB")
            bv_sb = wpool.tile([1, C], dt.bfloat16, tag="bv")
            bo_sb = wpool.tile([1, C], dt.bfloat16, tag="bo")
            for sb, d in (
                (bq_sb, bq_d), (bk_sb, bk_d), (gns_sb, gns_d), (gnb_sb, gnb_d),
                (sel_sb, sel_d), (selB_sb, selB_d), (bv_sb, bv_d), (bo_sb, bo_d),
            ):
                nc.sync.dma_start(sb[:], d[:])
            ones_col = wpool.tile([P, 1], dt.bfloat16, tag="ones_col")
            ones_r128b = wpool.tile([1, P], dt.bfloat16, tag="ones_r128b")
            ones_r512b = wpool.tile([1, C], dt.bfloat16, tag="ones_r512b")
            ones_r128f = wpool.tile([1, P], dt.float32, tag="ones_r128f")
            for t in (ones_col, ones_r128b, ones_r512b, ones_r128f):
                nc.vector.memset(t[:], 1.0)
            eps_sb = wpool.tile([8, 1], dt.float32, tag="eps")
            nc.vector.memset(eps_sb[:], EPS)

            # ---- GroupNorm: per c-tile stats -> per-channel scale/bias -> h ----
            h = []
            sq = sqpool.tile([P, N], dt.bfloat16, tag="sq")
            for t in range(CT):
                x_t = xpool.tile([P, N], dt.bfloat16, tag="x")
                nc.sync.dma_start(x_t[:], xbf_d[t * P:(t + 1) * P, :])
                s_t = gpool.tile([P, 2], dt.float32, tag="s")
                nc.vector.reduce_sum(out=s_t[:, 0:1], in_=x_t[:], axis=AX.X)
                nc.scalar.activation(sq[:], x_t[:], AF.Square, accum_out=s_t[:, 1:2])
                psg = psA.tile([8, 2], dt.float32, tag="mm")
                nc.tensor.matmul(psg[:], sel_sb[:], s_t[:], start=True, stop=True)
                gm = gpool.tile([8, 2], dt.float32, tag="gm")
                nc.vector.tensor_scalar_mul(gm[:], psg[:], 1.0 / (16 * N))
                msq = gpool.tile([8, 1], dt.float32, tag="msq")
                nc.vector.tensor_mul(msq[:], gm[:, 0:1], gm[:, 0:1])
                var = gpool.tile([8, 1], dt.float32, tag="var")
                nc.vector.tensor_sub(var[:], gm[:, 1:2], msq[:])
                sd = gpool.tile([8, 1], dt.float32, tag="sd")
                nc.scalar.activation(sd[:], var[:], AF.Sqrt, bias=eps_sb[:])
                rs2 = gpool.tile([8, 2], dt.float32, tag="rs2")
                nc.vector.reciprocal(rs2[:, 0:1], sd[:])
                nc.vector.tensor_mul(rs2[:, 1:2], gm[:, 0:1], rs2[:, 0:1])
                psb = psA.tile([P, 2], dt.float32, tag="mm")
                nc.tensor.matmul(psb[:], selB_sb[:], rs2[:], start=True, stop=True)
                chs = chpool.tile([P, 2], dt.float32, tag="chs")
                nc.vector.tensor_mul(chs[:, 0:1], psb[:, 0:1], gns_sb[:, t:t + 1])
                tmpb = chpool.tile([P, 1], dt.float32, tag="tmpb")
                nc.vector.tensor_scalar(
                    tmpb[:], psb[:, 1:2], gns_sb[:, t:t + 1], -1.0,
                    op0=ALU.mult, op1=ALU.mult,
                )
                nc.vector.tensor_add(chs[:, 1:2], tmpb[:], gnb_sb[:, t:t + 1])
                h_t = hpool.tile([P, N], dt.bfloat16, tag="h")
                nc.scalar.activation(
                    h_t[:], x_t[:], AF.Identity, bias=chs[:, 1:2], scale=chs[:, 0:1]
                )
                h.append(h_t)

            # ---- q, k projections (channels-major), biases via ACT copy ----
            q = []
            for m in range(CT):
                q_m = qpool.tile([P, NQ], dt.bfloat16, tag="q")
                q.append(q_m)
                for nb in range(QB):
                    psq = po.tile([P, NB], dt.float32, tag="acc")
                    for t in range(CT):
                        nc.tensor.matmul(
                            psq[:],
                            wq_sb[:, t, m * P:(m + 1) * P],
                            h[t][:, nb * NB:(nb + 1) * NB],
                            start=(t == 0), stop=(t == CT - 1),
                        )
                    nc.scalar.activation(
                        q_m[:, nb * NB:(nb + 1) * NB], psq[:], AF.Identity,
                        bias=bq_sb[:, m:m + 1],
                    )
            k = []
            for m in range(CT):
                k_m = kpool.tile([P, N], dt.bfloat16, tag="k")
                k.append(k_m)
                for nb in range(N // NB):
                    psk = po.tile([P, NB], dt.float32, tag="acc")
                    for t in range(CT):
                        nc.tensor.matmul(
                            psk[:],
                            wk_sb[:, t, m * P:(m + 1) * P],
                            h[t][:, nb * NB:(nb + 1) * NB],
                            start=(t == 0), stop=(t == CT - 1),
                        )
                    nc.scalar.activation(
                        k_m[:, nb * NB:(nb + 1) * NB], psk[:], AF.Identity,
                        bias=bk_sb[:, m:m + 1],
                    )

            # ---- v in tokens-major layout [Nk, C], bias via K=1 matmul ----
            v = []
            for nkt in range(NKT):
                psv = po.tile([P, C], dt.float32, tag="acc")
                for t in range(CT):
                    nc.tensor.matmul(
                        psv[:],
                        h[t][:, nkt * P:(nkt + 1) * P],
                        wv_sb[:, t, :],
                        start=(t == 0), stop=False,
                    )
                nc.tensor.matmul(psv[:], ones_r128b[:], bv_sb[:], start=False, stop=True)
                v_t = vpool.tile([P, C], dt.bfloat16, tag="v")
                nc.vector.tensor_copy(v_t[:], psv[:])
                v.append(v_t)

            # ---- attention + projection per 512-wide query block ----
            o = [opool.tile([P, NQ], dt.bfloat16, tag="o", name=f"o{ct}") for ct in range(CT)]
            for qb in range(QB):
                psden = pden.tile([1, NB], dt.float32, tag="den")
                po_c = [po.tile([P, NB], dt.float32, tag="acc", name=f"po{qb}_{ct}") for ct in range(CT)]

                def den_o(p_t, idx):
                    st, sp = idx == 0, idx == NKT - 1
                    nc.tensor.matmul(psden[:], ones_col[:], p_t[:], start=st, stop=sp)
                    for ct in range(CT):
                        nc.tensor.matmul(
                            po_c[ct][:], v[idx][:, ct * P:(ct + 1) * P], p_t[:],
                            start=st, stop=sp,
                        )

                prev = None
                for nkt in range(NKT):
                    pss = psA.tile([P, NB], dt.float32, tag="mm")
                    for m in range(CT):
                        nc.tensor.matmul(
                            pss[:],
                            k[m][:, nkt * P:(nkt + 1) * P],
                            q[m][:, qb * NB:(qb + 1) * NB],
                            start=(m == 0), stop=(m == CT - 1),
                        )
                    p_t = ppool.tile([P, NB], dt.bfloat16, tag="p")
                    nc.scalar.activation(p_t[:], pss[:], AF.Exp, scale=SCALE)
                    # software-pipeline: emit prev tile's den/o matmuls after
                    # this tile's scores so PE never stalls on the exp
                    if prev is not None:
                        den_o(prev, nkt - 1)
                    prev = p_t
                den_o(prev, NKT - 1)

                rden = gpool.tile([1, NB], dt.float32, tag="rden")
                nc.vector.reciprocal(rden[:], psden[:])
                psrb = pden.tile([P, NB], dt.float32, tag="rb")
                nc.tensor.matmul(psrb[:], ones_r128f[:], rden[:], start=True, stop=True)
                rb_sb = rbpool.tile([P, NB], dt.float32, tag="rb_sb")
                nc.vector.tensor_copy(rb_sb[:], psrb[:])
                for ct in range(CT):
                    nc.vector.tensor_mul(
                        o[ct][:, qb * NB:(qb + 1) * NB], po_c[ct][:], rb_sb[:]
                    )

                for m in range(CT):
                    psp = psA.tile([P, NB], dt.float32, tag="mm")
                    for ct in range(CT):
                        nc.tensor.matmul(
                            psp[:],
                            wo_sb[:, ct, m * P:(m + 1) * P],
                            o[ct][:, qb * NB:(qb + 1) * NB],
                            start=(ct == 0), stop=False,
                        )
                    nc.tensor.matmul(
                        psp[:], bo_sb[:, m * P:(m + 1) * P], ones_r512b[:],
                        start=False, stop=True,
                    )
                    xr = xrpool.tile([P, NB], dt.float32, tag="xr")
                    nc.sync.dma_start(
                        xr[:], xf32_d[m * P:(m + 1) * P, qb * NB:(qb + 1) * NB]
                    )
                    outt = outpool.tile([P, NB], dt.float32, tag="outt")
                    nc.vector.tensor_add(outt[:], psp[:], xr[:])
                    nc.sync.dma_start(
                        y_d[m * P:(m + 1) * P, qb * NB:(qb + 1) * NB], outt[:]
                    )

    _legalize_sync_waits(nc)
    return nc


def _host_inputs(x, gn_scale, gn_bias, wq, bq, wk, bk, wv, bv, wo, bo):
    xf = np.asarray(x, np.float32).reshape(4, C, N)

    def wprep(w):
        wT = np.asarray(w, np.float32).T  # [cin, cout]
        return np.ascontiguousarray(wT.reshape(CT, P, C).transpose(1, 0, 2)).astype(BF16)

    def colprep(b):
        return np.ascontiguousarray(np.asarray(b, np.float32).reshape(CT, P).T)

    sel = np.zeros((P, 8), np.float32)
    sel[np.arange(P), np.arange(P) // 16] = 1.0
    shared = {
        "wq": wprep(wq), "wk": wprep(wk), "wv": wprep(wv), "wo": wprep(wo),
        "bq": colprep(bq), "bk": colprep(bk),
        "bv": np.asarray(bv, np.float32).reshape(1, C).astype(BF16),
        "bo": np.asarray(bo, np.float32).reshape(1, C).astype(BF16),
        "gns": colprep(gn_scale), "gnb": colprep(gn_bias),
        "sel": sel, "selB": np.ascontiguousarray(sel.T),
    }
    in_maps = []
    for i in range(N_CORES):
        b, hf = i // 2, i % 2
        xs = xf[b] if hf == 0 else np.concatenate(
            [xf[b][:, NQ:], xf[b][:, :NQ]], axis=1
        )
        xs = np.ascontiguousarray(xs, dtype=np.float32)
        in_maps.append({"xf32": xs, "xbf": xs.astype(BF16), **shared})
    return in_maps


def kernel(x, gn_scale, gn_bias, wq, bq, wk, bk, wv, bv, wo, bo):
    global _LAST_RESULT
    from concourse.bass_utils import run_bass_kernel_spmd

    nc = _build_program()
    in_maps = _host_inputs(x, gn_scale, gn_bias, wq, bq, wk, bk, wv, bv, wo, bo)
    res = run_bass_kernel_spmd(
        nc, in_maps, list(range(N_CORES)),
        trace=bool(os.environ.get("KERNEL_TRACE")),
    )
    _LAST_RESULT = res
    full = np.empty((4, C, N), np.float32)
    for i in range(N_CORES):
        b, hf = i // 2, i % 2
        ycore = res.results[i]["y"]
        if hf == 0:
            full[b][:, :NQ] = ycore
        else:
            full[b][:, NQ:] = ycore
    return full.reshape(np.asarray(x).shape).astype(np.float32)
